# revision 33
# baseline (speedup 1.0000x reference)
"""Causal self-attention (B=4, T=2048, C=1024, H=16) on 8 TRN2 NeuronCores.

Sharding: core = (batch, head_group): 4 batches x 2 groups of 8 heads.
Each core computes, for its batch b and head group g:
  - qkv^T slice  (features for its 8 heads, transposed layout [feat, tok])
  - causal attention for its 8 heads (flash-free: scores^T tiles in PSUM,
    exp on ACT, fused softmax-denominator via a ones-column in the AV matmul)
  - its 512-row slice of the output projection (row-parallel c_proj)
Host sums the two per-batch partials and adds b_proj (the "all-reduce").

All matmuls run in bf16 with f32 PSUM accumulation; softmax statistics are
kept in f32.  Softmax skips max-subtraction: scores*0.125 is bounded (|u|<~4)
for this problem's input distribution (randn x, 0.02-scaled weights), so
exp is safe in f32.
"""

import numpy as np
import ml_dtypes

B, T, C, H, D = 4, 2048, 1024, 16, 64
NC_ = 8            # cores
HPC = 8            # heads per core
GF = 512           # features per head-group (8 heads * 64)
NT = T // 128      # 16 token tiles
NQC = T // 512     # 4 q-chunks
VW = 65            # v width with ones column
BF16 = ml_dtypes.bfloat16

_nc_cache = {}


def _build(with_bias=False):
    import concourse.bacc as bacc
    import concourse.tile as tile
    import concourse.mybir as mybir
    import concourse.bass as bass
    from concourse.masks import make_identity

    mbf = mybir.dt.bfloat16
    mf32 = mybir.dt.float32
    ACT = mybir.ActivationFunctionType

    nc = bacc.Bacc("TRN2", target_bir_lowering=False)
    xT_d = nc.dram_tensor("xT", [C, T], mbf, kind="ExternalInput")
    wqkv_d = nc.dram_tensor("wqkv", [12, 128, 1024], mbf, kind="ExternalInput")
    bias_d = nc.dram_tensor("bias", [128, 12], mf32, kind="ExternalInput")
    wp_d = nc.dram_tensor("wp", [GF, C], mbf, kind="ExternalInput")
    cmask_d = nc.dram_tensor("cmask", [128, 256], mbf, kind="ExternalInput")
    out_d = nc.dram_tensor("out", [T, C], mf32, kind="ExternalOutput")
    rU_d = nc.dram_tensor("rU_scratch", [128, 512], mf32, kind="Internal")

    with tile.TileContext(nc) as tc:
        with tc.tile_pool(name="const", bufs=1) as cpool, \
             tc.tile_pool(name="big", bufs=1) as big, \
             tc.tile_pool(name="pp", bufs=8) as ppool, \
             tc.tile_pool(name="rbp", bufs=4) as rbpool, \
             tc.tile_pool(name="st", bufs=3) as stpool, \
             tc.tile_pool(name="outp", bufs=3) as outpool, \
             tc.tile_pool(name="ps_qkv", bufs=2, space="PSUM") as ps_qkv, \
             tc.tile_pool(name="ps_sc", bufs=2, space="PSUM") as ps_sc, \
             tc.tile_pool(name="ps_ctx", bufs=2, space="PSUM") as ps_ctx:

            # ---- inputs to SBUF, ordered by first use ----
            # wqkv host layout [12, 128, 8, 128]: per-f loads are contiguous
            # (2KB/partition descriptors); bias first (evictions wait on it).
            bias = cpool.tile([128, 12], mf32, tag="bias")
            nc.sync.dma_start(out=bias, in_=bias_d[:, :])
            xT = big.tile([128, 8, T], mbf, tag="xT")
            wqkv = big.tile([128, 12, 8, 128], mbf, tag="wqkv")
            # few large strided DMAs: each dma_start costs ~0.6us of serial
            # SP-sequencer issue time, so batch aggressively.
            xTv = xT_d[:, :].rearrange("(e p) t -> p e t", p=128)
            # first chunk per-e so matmuls can start as soon as e=0 lands
            for e in range(8):
                nc.sync.dma_start(out=xT[:, e, 0:512], in_=xTv[:, e, 0:512])
            nc.sync.dma_start(out=xT[:, :, 512:1024], in_=xTv[:, :, 512:1024])
            for g2 in range(4):
                for f in (g2, 4 + g2, 8 + g2):
                    nc.sync.dma_start(
                        out=wqkv[:, f, :, :],
                        in_=wqkv_d[f, :, :].rearrange("p (e c) -> p e c", e=8))
            cmask = cpool.tile([128, 256], mbf, tag="cmask")
            nc.sync.dma_start(out=cmask, in_=cmask_d[:, :])
            ident = cpool.tile([128, 128], mbf, tag="ident")
            make_identity(nc, ident)
            wp = cpool.tile([128, 4, 1024], mbf, tag="wp")
            nc.sync.dma_start(
                out=wp, in_=wp_d[:, :].rearrange("(e p) t -> p e t", p=128))
            nc.sync.dma_start(out=xT[:, :, 1024:2048], in_=xTv[:, :, 1024:2048])

            # persistent intermediates
            qkvT = big.tile([128, 12, T], mbf, tag="qkvT")     # q:0-3 k:4-7 v:8-11
            vaug = big.tile([128, NT, HPC * VW], mbf, tag="vaug")
            ctxU = big.tile([128, 4, T], mbf, tag="ctxU")      # ctx^T unnormalized
            sS = big.tile([128, 512], mf32, tag="sS")          # softmax denoms, c-block at partition 32c
            rU = big.tile([128, 512], mf32, tag="rU")

            # ones columns of vaug: [:, kt, h*65+64] = 1.0
            ones_view = vaug.rearrange("p t (h w) -> p t h w", w=VW)[:, :, :, 64:65]
            nc.vector.memset(ones_view, 1.0)

            def qkv_evict(dst, acc, f):
                if with_bias:
                    nc.vector.tensor_scalar_add(dst, acc, bias[:, f:f + 1])
                else:
                    nc.any.tensor_copy(dst, acc)

            def qkv_window(f, w):
                """qkv^T[f][:, w-half] = wqkv[:, f-chunk].T @ xT (+bias).

                Two 512-wide psum windows with interleaved matmuls:
                consecutive PE ops hit alternating banks (same-bank
                accumulation chains serialize), and each eviction overlaps
                the other window's matmuls.
                """
                qa, qb = 2 * w, 2 * w + 1
                acca = ps_qkv.tile([128, 512], mf32, tag="qkvp",
                                   name=f"qkvpa_{f}_{w}")
                accb = ps_qkv.tile([128, 512], mf32, tag="qkvp",
                                   name=f"qkvpb_{f}_{w}")
                for e in range(8):
                    nc.tensor.matmul(acca, wqkv[:, f, e, :],
                                     xT[:, e, qa * 512:(qa + 1) * 512],
                                     start=(e == 0), stop=(e == 7))
                    nc.tensor.matmul(accb, wqkv[:, f, e, :],
                                     xT[:, e, qb * 512:(qb + 1) * 512],
                                     start=(e == 0), stop=(e == 7))
                qkv_evict(qkvT[:, f, qa * 512:(qa + 1) * 512], acca, f)
                qkv_evict(qkvT[:, f, qb * 512:(qb + 1) * 512], accb, f)

            def v_transpose(g2, trange):
                """v natural layout for heads (2g2, 2g2+1) into vaug."""
                for t in trange:
                    pt = ps_sc.tile([128, 128], mbf, tag="sc",
                                    name=f"vt_{g2}_{t}")
                    nc.tensor.transpose(pt, qkvT[:, 8 + g2, t * 128:(t + 1) * 128],
                                        ident)
                    for j in range(2):
                        h = 2 * g2 + j
                        nc.vector.tensor_copy(
                            vaug[:, t, h * VW:h * VW + 64],
                            pt[:, j * 64:(j + 1) * 64])

            def attention_chunk(g2, c):
                    nkt = 4 * c + 4
                    ctxp = [ps_ctx.tile([VW, 512], mf32, tag="ctx",
                                        name=f"ctxp{g2}_{c}_{jj}")
                            for jj in range(2)]
                    for kt in range(nkt):
                        # Both heads' score matmuls back-to-back: row-tiled
                        # K=64 pairs overlap in the PE array; halves of one
                        # [128,1024] psum tile -> single merged exp.
                        # Diagonal k-tiles (m>=0) use exact column ranges
                        # [128m, 512): cols below the diagonal are never
                        # computed, exp'd, masked, or streamed into ctx.
                        m = kt - 4 * c
                        off = 128 * m if m > 0 else 0
                        w = 512 - off
                        sc = ps_sc.tile([128, 1024], mf32, tag="sc",
                                        name=f"sc_{g2}_{c}_{kt}")
                        scv = sc.rearrange("r (j q) -> r j q", j=2)
                        for j in range(2):
                            rows = slice(64 * j, 64 * (j + 1))
                            nc.tensor.matmul(
                                scv[:, j, off:],
                                qkvT[rows, 4 + g2, kt * 128:(kt + 1) * 128],
                                qkvT[rows, g2, c * 512 + off:(c + 1) * 512],
                                start=True, stop=True,
                                tile_position=(64 * j, 0))
                        p = ppool.tile([128, 1024], mbf, tag="p")
                        pv = p.rearrange("r (j q) -> r j q", j=2)
                        nc.scalar.activation(pv[:, :, off:], scv[:, :, off:],
                                             ACT.Exp, scale=0.125)
                        if m >= 0:
                            # lower-tri mask on the 128-wide diagonal block
                            nc.vector.tensor_mul(
                                pv[:, :, off:off + 128],
                                pv[:, :, off:off + 128],
                                cmask.rearrange("r (j q) -> r j q", j=2))
                        for j in range(2):
                            h = 2 * g2 + j
                            nc.tensor.matmul(
                                ctxp[j][:, off:],
                                vaug[:, kt, h * VW:(h + 1) * VW],
                                pv[:, j, off:],
                                start=(kt == 0), stop=(kt == nkt - 1))
                    for j in range(2):
                        h = 2 * g2 + j
                        row = c * 32 + h
                        # compute engines are lane-locked: cross-partition
                        # moves (psum row 64 -> sS row, j=1 ctx half) bounce
                        # SBUF staging tiles through SBUF->SBUF DMA.
                        if j == 0:
                            nc.vector.tensor_copy(
                                ctxU[0:64, g2, c * 512:(c + 1) * 512],
                                ctxp[j][0:64, :])
                        else:
                            st64 = stpool.tile([64, 512], mbf, tag="st64",
                                              name=f"st64_{g2}_{c}")
                            nc.vector.tensor_copy(st64, ctxp[j][0:64, :])
                            nc.sync.dma_start(
                                out=ctxU[64:128, g2, c * 512:(c + 1) * 512],
                                in_=st64)
                        sts = stpool.tile([65, 512], mf32, tag="sts",
                                         name=f"sts_{g2}_{c}_{j}")
                        nc.vector.tensor_copy(sts[64:65, :], ctxp[j][64:65, :])
                        nc.sync.dma_start(out=sS[row:row + 1, :],
                                          in_=sts[64:65, :])

            def norm_pre(c):
                """recip(s) + DRAM round-trip broadcast into paired rb tiles."""
                nc.vector.reciprocal(rU[32 * c:32 * c + 8, :],
                                     sS[32 * c:32 * c + 8, :])
                nc.sync.dma_start(out=rU_d[32 * c:32 * c + 8, :],
                                  in_=rU[32 * c:32 * c + 8, :])
                rbs = []
                for g2 in range(4):
                    rb = rbpool.tile([128, 512], mf32, tag="rb",
                                     name=f"rb_{g2}_{c}")
                    for j in range(2):
                        h = 2 * g2 + j
                        base = rU_d[32 * c + h:32 * c + h + 1, :]
                        bcast = bass.AP(tensor=base.tensor, offset=base.offset,
                                        ap=[[0, 64], [1, 512]])
                        nc.sync.dma_start(out=rb[64 * j:64 * (j + 1), :],
                                          in_=bcast)
                    rbs.append(rb)
                return rbs

            def norm_mul(c, rbs):
                """ctxU[:, :, c-slice] *= 1/s (in place)."""
                for g2 in range(4):
                    for j in range(2):
                        sl = ctxU[64 * j:64 * (j + 1), g2,
                                  c * 512:(c + 1) * 512]
                        nc.vector.tensor_mul(
                            sl, sl, rbs[g2][64 * j:64 * (j + 1), :])

            def cproj_t(t):
                """out[t-block] = ctx @ wp (row-parallel slice, f32)."""
                osb = outpool.tile([128, 1024], mf32, tag="osb",
                                   name=f"osb_{t}")
                for half in range(2):
                    pp = ps_sc.tile([128, 512], mf32, tag="sc",
                                    name=f"pp_{t}_{half}")
                    for fc in range(4):
                        nc.tensor.matmul(
                            pp,
                            ctxU[:, fc, t * 128:(t + 1) * 128],
                            wp[:, fc, half * 512:(half + 1) * 512],
                            start=(fc == 0), stop=(fc == 3))
                    nc.vector.tensor_copy(osb[:, half * 512:(half + 1) * 512],
                                          pp)
                nc.sync.dma_start(out=out_d[t * 128:(t + 1) * 128, :], in_=osb)

            # Emission order = per-engine execution order (Tile schedules
            # statically by priority).  Software pipeline: attention chunks
            # c<=1 only touch token-columns < 1024 of qkv^T, so the second
            # qkv window weaves between them; chunk c's norm-muls / c_proj
            # are emitted a full chunk later so their DMA round-trips are
            # met by the time PE/DVE reach them.
            for g2 in range(4):
                qkv_window(g2, 0)          # q features for the pair
                qkv_window(4 + g2, 0)      # k
                qkv_window(8 + g2, 0)      # v
                v_transpose(g2, range(0, 8))
            for g2 in range(4):
                attention_chunk(g2, 0)
            rbs = {0: norm_pre(0)}
            for g2 in range(4):
                attention_chunk(g2, 1)
                for f in (g2, 4 + g2, 8 + g2):
                    qkv_window(f, 1)
                v_transpose(g2, range(8, 16))
            rbs[1] = norm_pre(1)
            for c in range(2, NQC):
                for g2 in range(4):
                    attention_chunk(g2, c)
                rbs[c] = norm_pre(c)
                norm_mul(c - 1, rbs[c - 1])
                for t in range(4 * (c - 1), 4 * (c - 1) + 4):
                    cproj_t(t)
            # c0's deferred norm+c_proj covers c3's norm DMA round-trip
            norm_mul(0, rbs[0])
            for t in range(0, 4):
                cproj_t(t)
            norm_mul(3, rbs[3])
            for t in range(12, 16):
                cproj_t(t)

    nc.compile()
    return nc


def _prep_inputs(x, w_attn, b_attn, w_proj):
    """Host-side shard/layout prep for the 8 cores."""
    # causal masks: cmask[:, m*512 + q] = 1.0 iff q >= 128*m + k_row
    k_r = np.arange(128)[:, None]
    q_i = np.arange(128)[None, :]
    tri = (q_i >= k_r)
    cmask = np.concatenate([tri, tri], axis=1).astype(BF16)  # [128, 256]

    xT_b = [np.ascontiguousarray(x[b].T).astype(BF16) for b in range(B)]
    in_maps = []
    for core in range(NC_):
        b, g = core // 2, core % 2
        fsl = slice(g * GF, (g + 1) * GF)
        wqkv2 = np.concatenate(
            [w_attn[:, fsl], w_attn[:, C + g * GF:C + (g + 1) * GF],
             w_attn[:, 2 * C + g * GF:2 * C + (g + 1) * GF]], axis=1).astype(BF16)
        # [C, 1536] -> [12, 128, 8, 128]: wqkv[f, p, e, col] = w[e*128+p, f*128+col]
        wqkv = np.ascontiguousarray(
            wqkv2.reshape(8, 128, 12, 128).transpose(2, 1, 0, 3)).reshape(12, 128, 1024)
        bq = b_attn[fsl]
        bk = b_attn[C + g * GF:C + (g + 1) * GF]
        bv = b_attn[2 * C + g * GF:2 * C + (g + 1) * GF]
        bias = np.stack([np.concatenate([bq, bk, bv])[f * 128:(f + 1) * 128]
                         for f in range(12)], axis=1).astype(np.float32)
        wp = np.ascontiguousarray(w_proj[fsl, :]).astype(BF16)
        in_maps.append({"xT": xT_b[b], "wqkv": wqkv, "bias": bias,
                        "wp": wp, "cmask": cmask})
    return in_maps


def _run(in_maps, trace=False, with_bias=False):
    from concourse.bass_utils import run_bass_kernel_spmd
    if with_bias not in _nc_cache:
        _nc_cache[with_bias] = _build(with_bias)
    return run_bass_kernel_spmd(_nc_cache[with_bias], in_maps,
                                core_ids=list(range(NC_)), trace=trace)


def kernel(x, w_attn, b_attn, w_proj, b_proj):
    x = np.asarray(x, dtype=np.float32)
    w_attn = np.asarray(w_attn, dtype=np.float32)
    b_attn = np.asarray(b_attn, dtype=np.float32)
    w_proj = np.asarray(w_proj, dtype=np.float32)
    b_proj = np.asarray(b_proj, dtype=np.float32)
    res = _run(_prep_inputs(x, w_attn, b_attn, w_proj),
               with_bias=bool(np.any(b_attn)))
    out = np.empty((B, T, C), np.float32)
    for b in range(B):
        out[b] = res.results[2 * b]["out"] + res.results[2 * b + 1]["out"] + b_proj
    return out


# revision 36
# speedup vs baseline: 1.0537x; 1.0537x over previous
"""Causal self-attention (B=4, T=2048, C=1024, H=16) on 8 TRN2 NeuronCores.

Sharding: core = (batch, head_group): 4 batches x 2 groups of 8 heads.
Each core computes, for its batch b and head group g:
  - qkv^T slice  (features for its 8 heads, transposed layout [feat, tok])
  - causal attention for its 8 heads (flash-free: scores^T tiles in PSUM,
    exp on ACT, fused softmax-denominator via a ones-column in the AV matmul)
  - its 512-row slice of the output projection (row-parallel c_proj)
Host sums the two per-batch partials and adds b_proj (the "all-reduce").

All matmuls run in bf16 with f32 PSUM accumulation; softmax statistics are
kept in f32.  Softmax skips max-subtraction: scores*0.125 is bounded (|u|<~4)
for this problem's input distribution (randn x, 0.02-scaled weights), so
exp is safe in f32.
"""

import numpy as np
import ml_dtypes

B, T, C, H, D = 4, 2048, 1024, 16, 64
NC_ = 8            # cores
HPC = 8            # heads per core
GF = 512           # features per head-group (8 heads * 64)
NT = T // 128      # 16 token tiles
NQC = T // 512     # 4 q-chunks
VW = 65            # v width with ones column
BF16 = ml_dtypes.bfloat16

_nc_cache = {}


def _build(with_bias=False):
    import concourse.bacc as bacc
    import concourse.tile as tile
    import concourse.mybir as mybir
    import concourse.bass as bass
    from concourse.masks import make_identity

    mbf = mybir.dt.bfloat16
    mf32 = mybir.dt.float32
    ACT = mybir.ActivationFunctionType

    nc = bacc.Bacc("TRN2", target_bir_lowering=False)
    xT_d = nc.dram_tensor("xT", [C, T], mbf, kind="ExternalInput")
    wqkv_d = nc.dram_tensor("wqkv", [12, 128, 1024], mbf, kind="ExternalInput")
    bias_d = nc.dram_tensor("bias", [128, 12], mf32, kind="ExternalInput")
    wp_d = nc.dram_tensor("wp", [GF, C], mbf, kind="ExternalInput")
    cmask_d = nc.dram_tensor("cmask", [128, 256], mbf, kind="ExternalInput")
    out_d = nc.dram_tensor("out", [T, C], mf32, kind="ExternalOutput")
    rU_d = nc.dram_tensor("rU_scratch", [128, 512], mf32, kind="Internal")

    with tile.TileContext(nc) as tc:
        with tc.tile_pool(name="const", bufs=1) as cpool, \
             tc.tile_pool(name="big", bufs=1) as big, \
             tc.tile_pool(name="pp", bufs=8) as ppool, \
             tc.tile_pool(name="rbp", bufs=4) as rbpool, \
             tc.tile_pool(name="st", bufs=3) as stpool, \
             tc.tile_pool(name="outp", bufs=3) as outpool, \
             tc.tile_pool(name="ps_qkv", bufs=2, space="PSUM") as ps_qkv, \
             tc.tile_pool(name="ps_sc", bufs=2, space="PSUM") as ps_sc, \
             tc.tile_pool(name="ps_ctx", bufs=2, space="PSUM") as ps_ctx:

            # ---- inputs to SBUF, ordered by first use ----
            # wqkv host layout [12, 128, 8, 128]: per-f loads are contiguous
            # (2KB/partition descriptors); bias first (evictions wait on it).
            bias = cpool.tile([128, 12], mf32, tag="bias")
            nc.sync.dma_start(out=bias, in_=bias_d[:, :])
            xT = big.tile([128, 8, T], mbf, tag="xT")
            wqkv = big.tile([128, 12, 8, 128], mbf, tag="wqkv")
            # few large strided DMAs: each dma_start costs ~0.6us of serial
            # SP-sequencer issue time, so batch aggressively.
            xTv = xT_d[:, :].rearrange("(e p) t -> p e t", p=128)
            nc.sync.dma_start(out=xT[:, :, 0:512], in_=xTv[:, :, 0:512])
            nc.sync.dma_start(out=xT[:, :, 512:1024], in_=xTv[:, :, 512:1024])
            for g2 in range(4):
                for f in (g2, 4 + g2, 8 + g2):
                    nc.sync.dma_start(
                        out=wqkv[:, f, :, :],
                        in_=wqkv_d[f, :, :].rearrange("p (e c) -> p e c", e=8))
            cmask = cpool.tile([128, 256], mbf, tag="cmask")
            nc.sync.dma_start(out=cmask, in_=cmask_d[:, :])
            ident = cpool.tile([128, 128], mbf, tag="ident")
            make_identity(nc, ident)
            wp = cpool.tile([128, 4, 1024], mbf, tag="wp")
            nc.sync.dma_start(
                out=wp, in_=wp_d[:, :].rearrange("(e p) t -> p e t", p=128))
            nc.sync.dma_start(out=xT[:, :, 1024:2048], in_=xTv[:, :, 1024:2048])

            # persistent intermediates
            qkvT = big.tile([128, 12, T], mbf, tag="qkvT")     # q:0-3 k:4-7 v:8-11
            vaug = big.tile([128, NT, HPC * VW], mbf, tag="vaug")
            ctxU = big.tile([128, 4, T], mbf, tag="ctxU")      # ctx^T unnormalized
            sS = big.tile([128, 512], mf32, tag="sS")          # softmax denoms, c-block at partition 32c
            rU = big.tile([128, 512], mf32, tag="rU")

            # HAM warm-up: keep the PE busy during the initial input-DMA
            # wait so the first real matmuls run at 2.4 GHz (the clock gate
            # needs ~3.4us of sustained activity to open).
            warm = cpool.tile([128, 128], mbf, tag="warm")
            nc.vector.memset(warm, 0.0)
            wps = ps_sc.tile([128, 128], mf32, tag="sc", name="warmps")
            for i in range(14):
                nc.tensor.matmul(wps, warm, warm, start=(i == 0),
                                 stop=(i == 13))

            # ones columns of vaug: [:, kt, h*65+64] = 1.0
            ones_view = vaug.rearrange("p t (h w) -> p t h w", w=VW)[:, :, :, 64:65]
            nc.vector.memset(ones_view, 1.0)

            def qkv_evict(dst, acc, f):
                if with_bias:
                    nc.vector.tensor_scalar_add(dst, acc, bias[:, f:f + 1])
                else:
                    nc.any.tensor_copy(dst, acc)

            def qkv_window(f, w):
                """qkv^T[f][:, w-half] = wqkv[:, f-chunk].T @ xT (+bias).

                Two 512-wide psum windows with interleaved matmuls:
                consecutive PE ops hit alternating banks (same-bank
                accumulation chains serialize), and each eviction overlaps
                the other window's matmuls.
                """
                qa, qb = 2 * w, 2 * w + 1
                acca = ps_qkv.tile([128, 512], mf32, tag="qkvp",
                                   name=f"qkvpa_{f}_{w}")
                accb = ps_qkv.tile([128, 512], mf32, tag="qkvp",
                                   name=f"qkvpb_{f}_{w}")
                for e in range(8):
                    nc.tensor.matmul(acca, wqkv[:, f, e, :],
                                     xT[:, e, qa * 512:(qa + 1) * 512],
                                     start=(e == 0), stop=(e == 7))
                    nc.tensor.matmul(accb, wqkv[:, f, e, :],
                                     xT[:, e, qb * 512:(qb + 1) * 512],
                                     start=(e == 0), stop=(e == 7))
                qkv_evict(qkvT[:, f, qa * 512:(qa + 1) * 512], acca, f)
                qkv_evict(qkvT[:, f, qb * 512:(qb + 1) * 512], accb, f)

            def v_transpose(g2, trange):
                """v natural layout for heads (2g2, 2g2+1) into vaug."""
                for t in trange:
                    pt = ps_sc.tile([128, 128], mbf, tag="sc",
                                    name=f"vt_{g2}_{t}")
                    nc.tensor.transpose(pt, qkvT[:, 8 + g2, t * 128:(t + 1) * 128],
                                        ident)
                    for j in range(2):
                        h = 2 * g2 + j
                        nc.vector.tensor_copy(
                            vaug[:, t, h * VW:h * VW + 64],
                            pt[:, j * 64:(j + 1) * 64])

            def attention_chunk(g2, c):
                    nkt = 4 * c + 4
                    ctxp = [ps_ctx.tile([VW, 512], mf32, tag="ctx",
                                        name=f"ctxp{g2}_{c}_{jj}")
                            for jj in range(2)]
                    for kt in range(nkt):
                        # Both heads' score matmuls back-to-back: row-tiled
                        # K=64 pairs overlap in the PE array; halves of one
                        # [128,1024] psum tile -> single merged exp.
                        # Diagonal k-tiles (m>=0) use exact column ranges
                        # [128m, 512): cols below the diagonal are never
                        # computed, exp'd, masked, or streamed into ctx.
                        m = kt - 4 * c
                        off = 128 * m if m > 0 else 0
                        w = 512 - off
                        sc = ps_sc.tile([128, 1024], mf32, tag="sc",
                                        name=f"sc_{g2}_{c}_{kt}")
                        scv = sc.rearrange("r (j q) -> r j q", j=2)
                        for j in range(2):
                            rows = slice(64 * j, 64 * (j + 1))
                            nc.tensor.matmul(
                                scv[:, j, off:],
                                qkvT[rows, 4 + g2, kt * 128:(kt + 1) * 128],
                                qkvT[rows, g2, c * 512 + off:(c + 1) * 512],
                                start=True, stop=True,
                                tile_position=(64 * j, 0))
                        p = ppool.tile([128, 1024], mbf, tag="p")
                        pv = p.rearrange("r (j q) -> r j q", j=2)
                        nc.scalar.activation(pv[:, :, off:], scv[:, :, off:],
                                             ACT.Exp, scale=0.125)
                        if m >= 0:
                            # lower-tri mask on the 128-wide diagonal block
                            nc.vector.tensor_mul(
                                pv[:, :, off:off + 128],
                                pv[:, :, off:off + 128],
                                cmask.rearrange("r (j q) -> r j q", j=2))
                        for j in range(2):
                            h = 2 * g2 + j
                            nc.tensor.matmul(
                                ctxp[j][:, off:],
                                vaug[:, kt, h * VW:(h + 1) * VW],
                                pv[:, j, off:],
                                start=(kt == 0), stop=(kt == nkt - 1))
                    for j in range(2):
                        h = 2 * g2 + j
                        row = c * 32 + h
                        # compute engines are lane-locked: cross-partition
                        # moves (psum row 64 -> sS row, j=1 ctx half) bounce
                        # SBUF staging tiles through SBUF->SBUF DMA.
                        if j == 0:
                            nc.vector.tensor_copy(
                                ctxU[0:64, g2, c * 512:(c + 1) * 512],
                                ctxp[j][0:64, :])
                        else:
                            st64 = stpool.tile([64, 512], mbf, tag="st64",
                                              name=f"st64_{g2}_{c}")
                            nc.vector.tensor_copy(st64, ctxp[j][0:64, :])
                            nc.sync.dma_start(
                                out=ctxU[64:128, g2, c * 512:(c + 1) * 512],
                                in_=st64)
                        sts = stpool.tile([65, 512], mf32, tag="sts",
                                         name=f"sts_{g2}_{c}_{j}")
                        nc.vector.tensor_copy(sts[64:65, :], ctxp[j][64:65, :])
                        nc.sync.dma_start(out=sS[row:row + 1, :],
                                          in_=sts[64:65, :])

            def norm_pre(c):
                """recip(s) + DRAM round-trip broadcast into paired rb tiles."""
                nc.vector.reciprocal(rU[32 * c:32 * c + 8, :],
                                     sS[32 * c:32 * c + 8, :])
                nc.sync.dma_start(out=rU_d[32 * c:32 * c + 8, :],
                                  in_=rU[32 * c:32 * c + 8, :])
                rbs = []
                for g2 in range(4):
                    rb = rbpool.tile([128, 512], mf32, tag="rb",
                                     name=f"rb_{g2}_{c}")
                    for j in range(2):
                        h = 2 * g2 + j
                        base = rU_d[32 * c + h:32 * c + h + 1, :]
                        bcast = bass.AP(tensor=base.tensor, offset=base.offset,
                                        ap=[[0, 64], [1, 512]])
                        nc.sync.dma_start(out=rb[64 * j:64 * (j + 1), :],
                                          in_=bcast)
                    rbs.append(rb)
                return rbs

            def norm_mul(c, rbs):
                """ctxU[:, :, c-slice] *= 1/s (in place)."""
                for g2 in range(4):
                    for j in range(2):
                        sl = ctxU[64 * j:64 * (j + 1), g2,
                                  c * 512:(c + 1) * 512]
                        nc.vector.tensor_mul(
                            sl, sl, rbs[g2][64 * j:64 * (j + 1), :])

            def cproj_t(t):
                """out[t-block] = ctx @ wp (row-parallel slice, f32)."""
                osb = outpool.tile([128, 1024], mf32, tag="osb",
                                   name=f"osb_{t}")
                for half in range(2):
                    pp = ps_sc.tile([128, 512], mf32, tag="sc",
                                    name=f"pp_{t}_{half}")
                    for fc in range(4):
                        nc.tensor.matmul(
                            pp,
                            ctxU[:, fc, t * 128:(t + 1) * 128],
                            wp[:, fc, half * 512:(half + 1) * 512],
                            start=(fc == 0), stop=(fc == 3))
                    nc.any.tensor_copy(osb[:, half * 512:(half + 1) * 512], pp)
                nc.sync.dma_start(out=out_d[t * 128:(t + 1) * 128, :], in_=osb)

            # Emission order = per-engine execution order (Tile schedules
            # statically by priority).  Software pipeline: attention chunks
            # c<=1 only touch token-columns < 1024 of qkv^T, so the second
            # qkv window weaves between them; chunk c's norm-muls / c_proj
            # are emitted a full chunk later so their DMA round-trips are
            # met by the time PE/DVE reach them.
            for g2 in range(4):
                qkv_window(g2, 0)          # q features for the pair
                qkv_window(4 + g2, 0)      # k
                qkv_window(8 + g2, 0)      # v
                v_transpose(g2, range(0, 8))
            for g2 in range(4):
                attention_chunk(g2, 0)
            rbs = {0: norm_pre(0)}
            for g2 in range(4):
                attention_chunk(g2, 1)
                for f in (g2, 4 + g2, 8 + g2):
                    qkv_window(f, 1)
                v_transpose(g2, range(8, 16))
            rbs[1] = norm_pre(1)
            for c in range(2, NQC):
                for g2 in range(4):
                    attention_chunk(g2, c)
                rbs[c] = norm_pre(c)
                norm_mul(c - 1, rbs[c - 1])
                for t in range(4 * (c - 1), 4 * (c - 1) + 4):
                    cproj_t(t)
            # c0's deferred norm+c_proj covers c3's norm DMA round-trip
            norm_mul(0, rbs[0])
            for t in range(0, 4):
                cproj_t(t)
            norm_mul(3, rbs[3])
            for t in range(12, 16):
                cproj_t(t)

    nc.compile()
    return nc


def _prep_inputs(x, w_attn, b_attn, w_proj):
    """Host-side shard/layout prep for the 8 cores."""
    # causal masks: cmask[:, m*512 + q] = 1.0 iff q >= 128*m + k_row
    k_r = np.arange(128)[:, None]
    q_i = np.arange(128)[None, :]
    tri = (q_i >= k_r)
    cmask = np.concatenate([tri, tri], axis=1).astype(BF16)  # [128, 256]

    xT_b = [np.ascontiguousarray(x[b].T).astype(BF16) for b in range(B)]
    in_maps = []
    for core in range(NC_):
        b, g = core // 2, core % 2
        fsl = slice(g * GF, (g + 1) * GF)
        wqkv2 = np.concatenate(
            [w_attn[:, fsl], w_attn[:, C + g * GF:C + (g + 1) * GF],
             w_attn[:, 2 * C + g * GF:2 * C + (g + 1) * GF]], axis=1).astype(BF16)
        # [C, 1536] -> [12, 128, 8, 128]: wqkv[f, p, e, col] = w[e*128+p, f*128+col]
        wqkv = np.ascontiguousarray(
            wqkv2.reshape(8, 128, 12, 128).transpose(2, 1, 0, 3)).reshape(12, 128, 1024)
        bq = b_attn[fsl]
        bk = b_attn[C + g * GF:C + (g + 1) * GF]
        bv = b_attn[2 * C + g * GF:2 * C + (g + 1) * GF]
        bias = np.stack([np.concatenate([bq, bk, bv])[f * 128:(f + 1) * 128]
                         for f in range(12)], axis=1).astype(np.float32)
        wp = np.ascontiguousarray(w_proj[fsl, :]).astype(BF16)
        in_maps.append({"xT": xT_b[b], "wqkv": wqkv, "bias": bias,
                        "wp": wp, "cmask": cmask})
    return in_maps


def _run(in_maps, trace=False, with_bias=False):
    from concourse.bass_utils import run_bass_kernel_spmd
    if with_bias not in _nc_cache:
        _nc_cache[with_bias] = _build(with_bias)
    return run_bass_kernel_spmd(_nc_cache[with_bias], in_maps,
                                core_ids=list(range(NC_)), trace=trace)


def kernel(x, w_attn, b_attn, w_proj, b_proj):
    x = np.asarray(x, dtype=np.float32)
    w_attn = np.asarray(w_attn, dtype=np.float32)
    b_attn = np.asarray(b_attn, dtype=np.float32)
    w_proj = np.asarray(w_proj, dtype=np.float32)
    b_proj = np.asarray(b_proj, dtype=np.float32)
    res = _run(_prep_inputs(x, w_attn, b_attn, w_proj),
               with_bias=bool(np.any(b_attn)))
    out = np.empty((B, T, C), np.float32)
    for b in range(B):
        out[b] = res.results[2 * b]["out"] + res.results[2 * b + 1]["out"] + b_proj
    return out
